# revision 21
# baseline (speedup 1.0000x reference)
"""Trainium2 Bass kernel for nn_AdaptiveReconstructionTransformer.

Pure data parallel over batch B=8 across 8 NeuronCores (one sample per core,
no collectives). Per-core the network runs out of SBUF with bf16 matmuls
(fp32 PSUM accumulation):

  - 1x1 convs / MLPs / qkv / proj: matmuls with channels on partitions.
  - 3x3 convs: 18 accumulating shifted matmuls over a zero-padded 34x34 image
    kept in SBUF; eval-mode BN folded into weights/bias host-side.
  - dynamic conv: kernel-bank mixing on the PE with a delta-structured
    (4*32 x 32) mixing matrix so mixed weights come out with the contraction
    dim on partitions, ready to be conv lhsT.
  - attention: q/k computed transposed (channel, token) so scores come out as
    S^T[t, s] with t on partitions; exp on the scalar engine (max-subtraction
    skipped: logits are O(10) in f32); V carries an interleaved ones column so
    each AV matmul also yields the softmax denominator row; per-head recip
    rows are broadcast over partitions with a tiny G matmul.
  - the per-head physics bias (pbias) is constant within each softmax row, so
    it cancels and is dropped.
"""

import sys

if "/opt/trn_rl_repo" not in sys.path:
    sys.path.insert(0, "/opt/trn_rl_repo")

from contextlib import ExitStack

import numpy as np
import ml_dtypes

import concourse.bass as bass
import concourse.mybir as mybir
import concourse.tile as tile
from concourse import bacc
from concourse.bass_utils import run_bass_kernel_spmd

BF16 = mybir.dt.bfloat16
F32 = mybir.dt.float32
AF = mybir.ActivationFunctionType
ALU = mybir.AluOpType
AX = mybir.AxisListType

B = 8
C = 256
S = 1024
HW = 32
PW = 34
PS = PW * PW
NH = 8
HD = 32
NK = 4
BN_SCALE = np.float32(1.0 / np.sqrt(1.0 + 1e-5))
EPS_GN = 1e-5

BF = ml_dtypes.bfloat16


def _np(x):
    return np.asarray(x, dtype=np.float32)


def _bf(x):
    return np.ascontiguousarray(np.asarray(x, dtype=np.float32).astype(BF))


def _f(x):
    return np.ascontiguousarray(np.asarray(x, dtype=np.float32))


def lhsT_1x1(Wm):
    """W (O, I) -> (128, I//128, O) with [p, kb, o] = W[o, kb*128+p]."""
    O, I = Wm.shape
    return np.ascontiguousarray(Wm.T.reshape(I // 128, 128, O).transpose(1, 0, 2))


def lhsT_3x3(Wc):
    """W (O, I, 3, 3) -> (128, I//128, 3, 3, O)."""
    O, I = Wc.shape[:2]
    t = Wc.transpose(1, 2, 3, 0)  # (I, ky, kx, O)
    t = t.reshape(I // 128, 128, 3, 3, O).transpose(1, 0, 2, 3, 4)
    return np.ascontiguousarray(t)


def bias_cols(b):
    """(O,) -> (128, O//128) with [p, ob] = b[ob*128+p]."""
    return np.ascontiguousarray(_np(b).reshape(-1, 128).T)


def prep_consts(params):
    cst = {}
    s = BN_SCALE

    fu = params["fusion"]
    for nm in ("pace", "adrn"):
        g = _np(fu[f"{nm}_bn_g"]) * s
        w = _np(fu[f"{nm}_w"])[:, :, 0, 0] * g[:, None]
        bb = _np(fu[f"{nm}_b"]) * g + _np(fu[f"{nm}_bn_b"])
        cst[f"fu_{nm}_w"] = _bf(lhsT_1x1(w))
        cst[f"fu_{nm}_b"] = _f(bias_cols(bb))
    aw = _np(fu["att_w"])[:, :, 0, 0]  # (2, 512)
    ab = _np(fu["att_b"])
    wd = aw[0] - aw[1]
    cst["fu_att_wp"] = _bf(lhsT_1x1(wd[:C][None, :]))  # (128, 2, 1)
    cst["fu_att_wa"] = _bf(lhsT_1x1(wd[C:][None, :]))
    cst["fu_att_bd"] = _f((ab[0] - ab[1]).reshape(1, 1))
    g = _np(fu["ref_bn_g"]) * s
    rw = _np(fu["ref_w"]) * g[:, None, None, None]
    rb = _np(fu["ref_b"]) * g + _np(fu["ref_bn_b"])
    cst["fu_ref_w"] = _bf(lhsT_3x3(rw))
    cst["fu_ref_b"] = _f(bias_cols(rb))

    for li, p in enumerate(params["dyn"]):
        g = _np(p["bn_g"]) * s
        bank = _np(p["bank"]) * g[None, :, None, None, None]  # (NK, O, I, 3, 3)
        t = bank.transpose(0, 2, 3, 4, 1)  # (n, I, ky, kx, O)
        t = t.reshape(NK, 2, 4, 32, 3, 3, C)  # (n, kb, j, ic, ky, kx, o)
        t = t.transpose(1, 2, 0, 3, 4, 5, 6)  # (kb, j, n, ic, ky, kx, o)
        cst[f"dy{li}_bank"] = _bf(t.reshape(2, 4, 128, 9 * C))
        cst[f"dy{li}_cb"] = _f(bias_cols(_np(p["bias"]) * g + _np(p["bn_b"])))
        cst[f"dy{li}_g1w"] = _bf(lhsT_1x1(_np(p["g1_w"])))  # (128, 2, 64)
        cst[f"dy{li}_g1b"] = _f(_np(p["g1_b"]).reshape(64, 1))
        cst[f"dy{li}_g2w"] = _bf(_np(p["g2_w"]).T)  # (64, 4)
        cst[f"dy{li}_g2b"] = _f(_np(p["g2_b"]).reshape(1, 4))

    for li, p in enumerate(params["tf"]):
        qkv = _np(p["qkv_w"])  # (768, 256)
        cst[f"tf{li}_qkw"] = _bf(lhsT_1x1(qkv[: 2 * C]))  # (128, 2, 512)
        cst[f"tf{li}_vw"] = _bf(lhsT_1x1(qkv[2 * C :]))  # (128, 2, 256)
        cst[f"tf{li}_pw"] = _bf(lhsT_1x1(_np(p["proj_w"])))
        cst[f"tf{li}_pb"] = _f(bias_cols(p["proj_b"]))
        cst[f"tf{li}_n1g"] = _f(bias_cols(p["n1_g"]))
        cst[f"tf{li}_n1b"] = _f(bias_cols(p["n1_b"]))
        cst[f"tf{li}_n2g"] = _f(bias_cols(p["n2_g"]))
        cst[f"tf{li}_n2b"] = _f(bias_cols(p["n2_b"]))
        cst[f"tf{li}_w1"] = _bf(lhsT_1x1(_np(p["mlp1_w"])))  # (128, 2, 1024)
        cst[f"tf{li}_b1"] = _f(bias_cols(p["mlp1_b"]))  # (128, 8)
        cst[f"tf{li}_w2"] = _bf(lhsT_1x1(_np(p["mlp2_w"])))  # (128, 8, 256)
        cst[f"tf{li}_b2"] = _f(bias_cols(p["mlp2_b"]))

    for li, p in enumerate(params["refine"]):
        g = _np(p["bn_g"]) * s
        cst[f"rf{li}_w"] = _bf(lhsT_3x3(_np(p["w"]) * g[:, None, None, None]))
        cst[f"rf{li}_b"] = _f(bias_cols(_np(p["b"]) * g + _np(p["bn_b"])))

    fi = params["final"]
    g = _np(fi["bn_g"]) * s
    cst["fin_w1"] = _bf(lhsT_3x3(_np(fi["w1"]) * g[:, None, None, None]))
    cst["fin_b1"] = _f(bias_cols(_np(fi["b1"]) * g + _np(fi["bn_b"])))
    cst["fin_w2"] = _bf(_np(fi["w2"])[:, :, 0, 0].T)  # (128, 1)
    cst["fin_b2"] = _f(_np(fi["b2"]).reshape(1, 1))

    cst["eye32x4"] = _bf(np.tile(np.eye(32, dtype=np.float32), (4, 1)))  # (128, 32)
    hmask = np.zeros((1, 512), np.float32)
    for j in range(4):
        hmask[0, 128 * j + 32 * j : 128 * j + 32 * (j + 1)] = 1.0
    cst["hmask"] = _bf(hmask)
    cst["ones11"] = _bf(np.ones((1, 1), np.float32))
    gt4 = np.zeros((4, 128), np.float32)
    for n in range(4):
        gt4[n, 32 * n : 32 * (n + 1)] = 1.0
    cst["gt4"] = _bf(gt4)
    ggrp = np.zeros((128, 16), np.float32)
    for p_ in range(128):
        ggrp[p_, p_ // 8] = 1.0 / 8.0
    cst["ggrp"] = _f(ggrp)
    gt16 = np.zeros((16, 128), np.float32)
    for p_ in range(128):
        gt16[p_ // 8, p_] = 1.0
    cst["gt16"] = _f(gt16)
    cst["ones1x128"] = _bf(np.ones((1, 128), np.float32))
    return cst


# ----------------------------------------------------------------------------


def build_graph(shapes, taps=()):
    taps = set(taps)
    nc = bacc.Bacc("TRN2", target_bir_lowering=False, debug=False, num_devices=B)

    def dt_of(a):
        return BF16 if a.dtype == BF else F32

    dram = {}
    for name, arr in shapes.items():
        dram[name] = nc.dram_tensor(name, arr.shape, dt_of(arr), kind="ExternalInput")
    out_d = nc.dram_tensor("out", (1, S), F32, kind="ExternalOutput")
    tap_d = {}

    def add_tap(name, shape, dtype=F32):
        if name in taps:
            tap_d[name] = nc.dram_tensor(
                f"tap_{name}", shape, dtype, kind="ExternalOutput"
            )
            return tap_d[name]
        return None

    with tile.TileContext(nc) as tc, ExitStack() as ctx:
        acts = ctx.enter_context(tc.tile_pool(name="acts", bufs=1))
        wp = ctx.enter_context(tc.tile_pool(name="wp", bufs=2))
        tp = ctx.enter_context(tc.tile_pool(name="tp", bufs=3))
        pp = ctx.enter_context(tc.tile_pool(name="pp", bufs=2, space="PSUM"))
        pps = ctx.enter_context(tc.tile_pool(name="pps", bufs=3, space="PSUM"))
        convA_cm = tc.tile_pool(name="convA", bufs=1)
        convA = convA_cm.__enter__()
        wmixp_cm = tc.tile_pool(name="wmixp", bufs=1)
        wmixp = wmixp_cm.__enter__()
        wbank_cm = tc.tile_pool(name="wbank", bufs=8)
        wbank = wbank_cm.__enter__()

        def load(name, pool=wp, tag=None):
            t = pool.tile(list(shapes[name].shape), dt_of(shapes[name]), tag=tag or name)
            nc.sync.dma_start(t[:], dram[name].ap())
            return t

        # persistent activations
        x = acts.tile([128, 2, S], F32, tag="x")
        pad_a = convA.tile([128, 2, PS], BF16, tag="pad_a")
        pad_b = convA.tile([128, 2, PS], BF16, tag="pad_b")

        nc.vector.memset(pad_a[:], 0.0)
        nc.vector.memset(pad_b[:], 0.0)

        eye32x4 = load("eye32x4", pool=acts)
        ones11 = load("ones11", pool=acts)
        gt4 = load("gt4", pool=acts)
        hmask = load("hmask", pool=acts)
        ggrp = load("ggrp", pool=acts)
        gt16 = load("gt16", pool=acts)
        ones1x128 = load("ones1x128", pool=acts)
        epsgn = acts.tile([16, 1], F32, tag="epsgn")
        nc.vector.memset(epsgn[:], EPS_GN)

        def interior(padt, kb):
            v = padt[:, kb, :].rearrange("p (h w) -> p h w", w=PW)
            return v[:, 1 : 1 + HW, 1 : 1 + HW]

        def pad_img(padt, kb):
            return padt[:, kb, :].rearrange("p (h w) -> p h w", w=PW)

        def conv3x3(src_pad, lhsT_fn, n_ob, writer):
            for ob in range(n_ob):
                for f in range(2):
                    ps = pp.tile([128, 512], F32, tag="ps")
                    first = True
                    for kb in range(2):
                        xv = pad_img(src_pad, kb)
                        for ky in range(3):
                            for kx in range(3):
                                nc.tensor.matmul(
                                    ps[:],
                                    lhsT_fn(kb, ky, kx, ob),
                                    xv[:, ky + 16 * f : ky + 16 * f + 16, kx : kx + 32],
                                    start=first,
                                    stop=(kb == 1 and ky == 2 and kx == 2),
                                )
                                first = False
                    writer(ob, f, ps)

        def relu_to_pad(dst_pad, bias_t):
            def w(ob, f, ps):
                dv = pad_img(dst_pad, ob)[:, 1 + 16 * f : 1 + 16 * f + 16, 1 : 1 + HW]
                nc.vector.tensor_scalar(
                    out=dv,
                    in0=ps[:].rearrange("p (a b) -> p a b", b=32),
                    scalar1=bias_t[:, ob : ob + 1],
                    scalar2=0.0,
                    op0=ALU.add,
                    op1=ALU.max,
                )
            return w

        def tap_pad(name, padt, nb=2):
            t = add_tap(name, (128, nb, HW * HW), BF16)
            if t is not None:
                for kb in range(nb):
                    nc.sync.dma_start(
                        t.ap().rearrange("p k (h w) -> p k h w", w=HW)[:, kb],
                        interior(padt, kb),
                    )

        def tap_x(name):
            t = add_tap(name, (128, 2, S))
            if t is not None:
                nc.sync.dma_start(t.ap(), x[:])

        # ==================== fusion ====================
        fus_cm = tc.tile_pool(name="fus", bufs=1)
        fus = fus_cm.__enter__()
        zin_p = load("z_pace", pool=fus)
        zin_a = load("z_adrn", pool=fus)
        zp_sb = fus.tile([128, 2, S], BF16, tag="zp")
        za_sb = fus.tile([128, 2, S], BF16, tag="za")
        w0sb = fus.tile([1, S], BF16, tag="w0sb")

        for nm, zin, zout in (("pace", zin_p, zp_sb), ("adrn", zin_a, za_sb)):
            wt = load(f"fu_{nm}_w")
            bt = load(f"fu_{nm}_b")
            for ob in range(2):
                for f in range(2):
                    ps = pp.tile([128, 512], F32, tag="ps")
                    for kb in range(2):
                        nc.tensor.matmul(
                            ps[:],
                            wt[:, kb, ob * 128 : (ob + 1) * 128],
                            zin[:, kb, 512 * f : 512 * (f + 1)],
                            start=(kb == 0),
                            stop=(kb == 1),
                        )
                    nc.scalar.activation(
                        zout[:, ob, 512 * f : 512 * (f + 1)],
                        ps[:],
                        AF.Relu,
                        bias=bt[:, ob : ob + 1],
                    )

        watp = load("fu_att_wp")
        wata = load("fu_att_wa")
        batd = load("fu_att_bd")
        for f in range(2):
            psg = pp.tile([128, 512], F32, tag="ps")
            k = 0
            for wt_, zt_ in ((watp, zp_sb), (wata, za_sb)):
                for kb in range(2):
                    nc.tensor.matmul(
                        psg[0:1, :],
                        wt_[:, kb, :],
                        zt_[:, kb, 512 * f : 512 * (f + 1)],
                        start=(k == 0),
                        stop=(k == 3),
                    )
                    k += 1
            nc.scalar.activation(
                w0sb[0:1, 512 * f : 512 * (f + 1)],
                psg[0:1, :],
                AF.Sigmoid,
                bias=batd[0:1, :],
            )
        for f in range(2):
            psw = pp.tile([128, 512], F32, tag="ps")
            nc.tensor.matmul(
                psw[:], ones1x128[:], w0sb[0:1, 512 * f : 512 * (f + 1)],
                start=True, stop=True,
            )
            for kb in range(2):
                t1 = tp.tile([128, 512], BF16, tag="gate_t1")
                nc.vector.tensor_sub(
                    t1[:],
                    zp_sb[:, kb, 512 * f : 512 * (f + 1)],
                    za_sb[:, kb, 512 * f : 512 * (f + 1)],
                )
                nc.vector.tensor_mul(t1[:], t1[:], psw[:])
                nc.vector.tensor_add(
                    pad_img(pad_a, kb)[:, 1 + 16 * f : 1 + 16 * f + 16, 1 : 1 + HW],
                    t1[:].rearrange("p (a b) -> p a b", b=32),
                    za_sb[:, kb, 512 * f : 512 * (f + 1)].rearrange(
                        "p (a b) -> p a b", b=32
                    ),
                )
        tap_pad("zf", pad_a)

        wt = load("fu_ref_w", tag="convw")
        bt = load("fu_ref_b", tag="convb")
        conv3x3(
            pad_a,
            lambda kb, ky, kx, ob, wt=wt: wt[:, kb, ky, kx, ob * 128 : (ob + 1) * 128],
            2,
            relu_to_pad(pad_b, bt),
        )
        fus_cm.__exit__(None, None, None)
        tap_pad("fus", pad_b)

        # ==================== dynamic convs ====================
        src, dst = pad_b, pad_a
        for li in range(3):
            g1w = load(f"dy{li}_g1w", tag="g1w")
            g1b = load(f"dy{li}_g1b", tag="g1b")
            g2w = load(f"dy{li}_g2w", tag="g2w")
            g2b = load(f"dy{li}_g2b", tag="g2b")
            cb = load(f"dy{li}_cb", tag="cb")

            gap = tp.tile([128, 2], F32, tag="gap")
            for kb in range(2):
                nc.vector.reduce_sum(gap[:, kb : kb + 1], interior(src, kb), axis=AX.XY)
            gapb = tp.tile([128, 2], BF16, tag="gapb")
            nc.vector.tensor_scalar_mul(gapb[:], gap[:], 1.0 / float(S))

            ps1 = pp.tile([128, 512], F32, tag="ps")
            for kb in range(2):
                nc.tensor.matmul(
                    ps1[0:64, 0:1], g1w[:, kb, :], gapb[:, kb : kb + 1],
                    start=(kb == 0), stop=(kb == 1),
                )
            h1 = tp.tile([64, 1], BF16, tag="h1")
            nc.scalar.activation(h1[:], ps1[0:64, 0:1], AF.Relu, bias=g1b[:])

            ps2 = pp.tile([128, 512], F32, tag="ps")
            nc.tensor.matmul(ps2[0:1, 0:4], h1[:], g2w[:], start=True, stop=True)
            lg = tp.tile([1, 4], F32, tag="lg")
            nc.vector.tensor_add(lg[:], ps2[0:1, 0:4], g2b[:])
            nc.scalar.activation(lg[:], lg[:], AF.Exp)
            lsum = tp.tile([1, 1], F32, tag="lsum")
            nc.vector.reduce_sum(lsum[:], lg[:], axis=AX.X)
            nc.vector.reciprocal(lsum[:], lsum[:])
            mixn = tp.tile([1, 4], BF16, tag="mixn")
            nc.vector.tensor_scalar_mul(mixn[:], lg[:], lsum[:])
            if f"mix{li}" in taps:
                t = add_tap(f"mix{li}", (1, 4))
                mixf = tp.tile([1, 4], F32, tag="mixf")
                nc.vector.tensor_copy(mixf[:], mixn[:])
                nc.sync.dma_start(t.ap(), mixf[:])

            psT = pp.tile([128, 512], F32, tag="ps")
            nc.tensor.matmul(psT[0:4, 0:1], mixn[:], ones11[:], start=True, stop=True)
            mixT = tp.tile([4, 1], BF16, tag="mixT")
            nc.vector.tensor_copy(mixT[:], psT[0:4, 0:1])
            psbc = pp.tile([128, 512], F32, tag="ps")
            nc.tensor.matmul(psbc[:, 0:1], gt4[:], mixT[:], start=True, stop=True)
            delta = tp.tile([128, 32], BF16, tag="delta")
            nc.vector.tensor_scalar_mul(delta[:], eye32x4[:], psbc[:, 0:1])

            wmix = wmixp.tile([128, 2, 9 * C], BF16, tag="wmix")
            for kb in range(2):
                bcs = []
                for j in range(4):
                    bc = wbank.tile([128, 9 * C], BF16, tag="bank")
                    nc.sync.dma_start(bc[:], dram[f"dy{li}_bank"].ap()[kb, j])
                    bcs.append(bc)
                off = 0
                while off < 9 * C:
                    cw = min(512, 9 * C - off)
                    psW = pp.tile([128, 512], F32, tag="ps")
                    for j in range(4):
                        nc.tensor.matmul(
                            psW[32 * j : 32 * (j + 1), :cw],
                            delta[:],
                            bcs[j][:, off : off + cw],
                            start=True,
                            stop=True,
                            tile_position=(0, 32 * j),
                        )
                    nc.vector.tensor_copy(wmix[:, kb, off : off + cw], psW[:, :cw])
                    off += cw

            conv3x3(
                src,
                lambda kb, ky, kx, ob, wmix=wmix: wmix[
                    :, kb, (ky * 3 + kx) * C + ob * 128 : (ky * 3 + kx) * C + (ob + 1) * 128
                ],
                2,
                relu_to_pad(dst, cb),
            )
            tap_pad(f"dyn{li}", dst)
            src, dst = dst, src

        # dyn0: b->a, dyn1: a->b, dyn2: b->a  =>  result in pad_a (== src)
        for kb in range(2):
            nc.vector.tensor_copy(
                x[:, kb, :].rearrange("p (h w) -> p h w", w=HW), interior(src, kb)
            )
        tap_x("x0")
        wbank_cm.__exit__(None, None, None)
        wmixp_cm.__exit__(None, None, None)
        convA_cm.__exit__(None, None, None)

        # ==================== transformer ====================
        convB = ctx.enter_context(tc.tile_pool(name="convB", bufs=1))
        pad_rf_a = convB.tile([128, 2, PS], BF16, tag="pad_a2")
        pad_rf_b = convB.tile([128, 2, PS], BF16, tag="pad_b2")
        hid = convB.tile([128, S], BF16, tag="hid")
        osb = convB.tile([1, S], F32, tag="osb")
        nc.vector.memset(pad_rf_a[:], 0.0)
        nc.vector.memset(pad_rf_b[:], 0.0)
        tfp_cm = tc.tile_pool(name="tfp", bufs=1)
        tfp = tfp_cm.__enter__()
        tfs_cm = tc.tile_pool(name="tfs", bufs=3)
        tfs = tfs_cm.__enter__()
        epool_cm = tc.tile_pool(name="epool", bufs=4)
        epool = epool_cm.__enter__()
        gnx = tfp.tile([128, 2, S], BF16, tag="gnx")
        qkT = tfp.tile([128, 4, S], BF16, tag="qkT")
        vaug = tfp.tile([128, 8, 8 * 64], BF16, tag="vaug")
        attnT = tfp.tile([128, 2, S], BF16, tag="attnT")
        hsb = tfp.tile([128, 8, S], BF16, tag="hsb")
        nc.vector.memset(
            vaug[:].rearrange("p tb (h e) -> p tb h e", e=64)[:, :, :, 32:64], 1.0
        )

        def groupnorm(gamma_t, beta_t, out_t):
            st4 = tp.tile([128, 4], F32, tag="st4")
            for kb in range(2):
                stats = tp.tile([128, 2, 6], F32, tag="gnstats")
                for i in range(2):
                    nc.vector.bn_stats(stats[:, i, :], x[:, kb, 512 * i : 512 * (i + 1)])
                nc.vector.bn_aggr(st4[:, 2 * kb : 2 * kb + 2], stats[:])
            tmp1 = tp.tile([128, 2], F32, tag="gn_t1")
            m_cols = st4[:].rearrange("p (k two) -> p k two", two=2)[:, :, 0]
            v_cols = st4[:].rearrange("p (k two) -> p k two", two=2)[:, :, 1]
            nc.vector.tensor_mul(tmp1[:], m_cols, m_cols)
            nc.vector.tensor_add(v_cols, v_cols, tmp1[:])
            psG = pp.tile([128, 512], F32, tag="ps")
            nc.tensor.matmul(psG[0:16, 0:4], ggrp[:], st4[:], start=True, stop=True)
            gst = tp.tile([16, 4], F32, tag="gst")
            nc.scalar.copy(gst[:], psG[0:16, 0:4])
            rs4 = tp.tile([16, 4], F32, tag="rs4")  # [var0, var1, m0, m1] -> rstd
            gm = gst[:].rearrange("p (k two) -> p k two", two=2)[:, :, 0]
            gv = gst[:].rearrange("p (k two) -> p k two", two=2)[:, :, 1]
            nc.vector.tensor_mul(rs4[:, 0:2], gm, gm)
            nc.vector.tensor_sub(rs4[:, 0:2], gv, rs4[:, 0:2])
            nc.scalar.copy(rs4[:, 2:4], gm)
            nc.scalar.activation(rs4[:, 0:2], rs4[:, 0:2], AF.Sqrt, bias=epsgn[:])
            nc.vector.reciprocal(rs4[:, 0:2], rs4[:, 0:2])
            psB = pp.tile([128, 512], F32, tag="ps")
            nc.tensor.matmul(psB[:, 0:4], gt16[:], rs4[:], start=True, stop=True)
            scl = tp.tile([128, 2], F32, tag="gn_scl")
            bsh = tp.tile([128, 2], F32, tag="gn_bsh")
            nc.vector.tensor_mul(scl[:], psB[:, 0:2], gamma_t[:])
            nc.vector.tensor_mul(bsh[:], psB[:, 2:4], scl[:])
            nc.vector.tensor_sub(bsh[:], beta_t[:], bsh[:])
            for kb in range(2):
                nc.scalar.activation(
                    out_t[:, kb, :], x[:, kb, :], AF.Identity,
                    bias=bsh[:, kb : kb + 1], scale=scl[:, kb : kb + 1],
                )

        for li in range(4):
            n1g = load(f"tf{li}_n1g", tag="n1g")
            n1b = load(f"tf{li}_n1b", tag="n1b")
            groupnorm(n1g, n1b, gnx)
            if li == 0 and "gnx0" in taps:
                t = add_tap("gnx0", (128, 2, S), BF16)
                nc.sync.dma_start(t.ap(), gnx[:])

            qkw = load(f"tf{li}_qkw", tag="qkw")
            for mb in range(4):
                for f in range(2):
                    ps = pp.tile([128, 512], F32, tag="ps")
                    for kb in range(2):
                        nc.tensor.matmul(
                            ps[:],
                            qkw[:, kb, mb * 128 : (mb + 1) * 128],
                            gnx[:, kb, 512 * f : 512 * (f + 1)],
                            start=(kb == 0),
                            stop=(kb == 1),
                        )
                    nc.vector.tensor_copy(qkT[:, mb, 512 * f : 512 * (f + 1)], ps[:])

            vw = load(f"tf{li}_vw", tag="vw")
            ver = vaug[:].rearrange("p tb (h e) -> p tb h e", e=64)
            for sb in range(8):
                ps = pp.tile([128, 512], F32, tag="ps")
                for kb in range(2):
                    nc.tensor.matmul(
                        ps[:, 0:256],
                        gnx[:, kb, sb * 128 : (sb + 1) * 128],
                        vw[:, kb, :],
                        start=(kb == 0),
                        stop=(kb == 1),
                    )
                nc.vector.tensor_copy(
                    ver[:, sb, :, 0:32],
                    ps[:, 0:256].rearrange("p (h e) -> p h e", e=32),
                )

            # attention, grouped by (q/k block, s-chunk): the 4 heads of a
            # block run their score matmuls interleaved across the 4 PE row
            # strips (concurrent in hardware); AV matmuls run col-packed two
            # heads at a time at array columns 0 and 64. Row 32/96 of each AV
            # psum is the softmax denominator (ones column in V); denominators
            # are broadcast over partitions with K=1 mask matmuls and inverted
            # with the fast approximate reciprocal.
            scale = float(HD) ** -0.5
            E_tiles = {}

            def emit_scores(kbh, sc):
                # two double-width E tiles: Epair[jp] holds heads 2*jp and 2*jp+1
                Es = [epool.tile([128, 8, 2, 512], BF16, tag="E", name=f"E_{kbh}_{sc}_{jj}") for jj in range(2)]
                E_tiles[(kbh, sc)] = Es
                for tb in range(8):
                    for jp in range(2):
                        psS = pps.tile([128, 1024], F32, tag="psS")
                        for idx in range(2):
                            j = 2 * jp + idx
                            nc.tensor.matmul(
                                psS[:, 512 * idx : 512 * (idx + 1)],
                                qkT[32 * j : 32 * (j + 1), 2 + kbh, tb * 128 : (tb + 1) * 128],
                                qkT[32 * j : 32 * (j + 1), kbh, 512 * sc : 512 * (sc + 1)],
                                start=True,
                                stop=True,
                                tile_position=(32 * j, 0),
                            )
                        if jp == 1 and tb % 2 == 0:
                            # Schraudolph-style exp on DVE+GpSimd to offload
                            # the scalar engine: bitcast(int32(A*scale*x + B))
                            # ~= exp(scale*x) to within +-3%; softmax
                            # normalization cancels the common-mode part.
                            yi = tfs.tile([128, 1024], mybir.dt.int32, tag="yi")
                            nc.vector.tensor_scalar(
                                out=yi[:],
                                in0=psS[:],
                                scalar1=12102203.1616 * scale,
                                scalar2=1064866805.0,
                                op0=ALU.mult,
                                op1=ALU.add,
                            )
                            nc.vector.tensor_copy(
                                Es[jp][:, tb],
                                yi[:].bitcast(F32).rearrange("p (a b) -> p a b", a=2),
                            )
                        else:
                            nc.scalar.activation(
                                Es[jp][:, tb], psS[:].rearrange("p (a b) -> p a b", a=2),
                                AF.Exp, scale=scale,
                            )

            def emit_av(kbh, sc):
                Es = E_tiles.pop((kbh, sc))
                for pair in ((0, 1), (2, 3)):
                    psA = pp.tile([128, 512], F32, tag="ps")
                    for tb in range(8):
                        for idx, j in enumerate(pair):
                            h = 4 * kbh + j
                            nc.tensor.matmul(
                                psA[64 * idx : 64 * (idx + 1), :],
                                vaug[:, tb, 64 * h : 64 * (h + 1)],
                                Es[j // 2][:, tb, j % 2],
                                start=(tb == 0),
                                stop=(tb == 7),
                                tile_position=(0, 64 * idx),
                            )
                    # rows [32:64) and [96:128) hold 32 copies of each head's
                    # softmax denominator (ones block in V); invert the whole
                    # tile in one pass (numerator lanes unused) and normalize.
                    rp = tfs.tile([128, 512], F32, tag="rp")
                    nc.vector.reciprocal_approx_fast(rp[:], psA[:])
                    for idx, j in enumerate(pair):
                        nc.vector.tensor_mul(
                            attnT[32 * j : 32 * (j + 1), kbh, 512 * sc : 512 * (sc + 1)],
                            psA[64 * idx : 64 * idx + 32, :],
                            rp[64 * idx + 32 : 64 * (idx + 1), :],
                        )

            groups = [(kbh, sc) for kbh in range(2) for sc in range(2)]
            emit_scores(*groups[0])
            for i, g in enumerate(groups):
                if i + 1 < len(groups):
                    emit_scores(*groups[i + 1])
                emit_av(*g)

            pw = load(f"tf{li}_pw", tag="pw")
            pb = load(f"tf{li}_pb", tag="pb")
            for ob in range(2):
                for f in range(2):
                    ps = pp.tile([128, 512], F32, tag="ps")
                    for kb in range(2):
                        nc.tensor.matmul(
                            ps[:],
                            pw[:, kb, ob * 128 : (ob + 1) * 128],
                            attnT[:, kb, 512 * f : 512 * (f + 1)],
                            start=(kb == 0),
                            stop=(kb == 1),
                        )
                    tb_ = tp.tile([128, 512], F32, tag="resid_t")
                    nc.scalar.activation(
                        tb_[:], ps[:], AF.Identity, bias=pb[:, ob : ob + 1]
                    )
                    nc.vector.tensor_add(
                        x[:, ob, 512 * f : 512 * (f + 1)],
                        x[:, ob, 512 * f : 512 * (f + 1)],
                        tb_[:],
                    )
            tap_x(f"attn{li}")

            n2g = load(f"tf{li}_n2g", tag="n2g")
            n2b = load(f"tf{li}_n2b", tag="n2b")
            groupnorm(n2g, n2b, gnx)
            w1 = load(f"tf{li}_w1", tag="w1")
            b1 = load(f"tf{li}_b1", tag="b1")
            for mb in range(8):
                for f in range(2):
                    ps = pp.tile([128, 512], F32, tag="ps")
                    for kb in range(2):
                        nc.tensor.matmul(
                            ps[:],
                            w1[:, kb, mb * 128 : (mb + 1) * 128],
                            gnx[:, kb, 512 * f : 512 * (f + 1)],
                            start=(kb == 0),
                            stop=(kb == 1),
                        )
                    nc.scalar.activation(
                        hsb[:, mb, 512 * f : 512 * (f + 1)], ps[:], AF.Gelu,
                        bias=b1[:, mb : mb + 1],
                    )
            w2 = load(f"tf{li}_w2", tag="w2")
            b2 = load(f"tf{li}_b2", tag="b2")
            for ob in range(2):
                for f in range(2):
                    ps = pp.tile([128, 512], F32, tag="ps")
                    for kb in range(8):
                        nc.tensor.matmul(
                            ps[:],
                            w2[:, kb, ob * 128 : (ob + 1) * 128],
                            hsb[:, kb, 512 * f : 512 * (f + 1)],
                            start=(kb == 0),
                            stop=(kb == 7),
                        )
                    tb_ = tp.tile([128, 512], F32, tag="resid_t")
                    nc.scalar.activation(
                        tb_[:], ps[:], AF.Identity, bias=b2[:, ob : ob + 1]
                    )
                    nc.vector.tensor_add(
                        x[:, ob, 512 * f : 512 * (f + 1)],
                        x[:, ob, 512 * f : 512 * (f + 1)],
                        tb_[:],
                    )
            tap_x(f"tf{li}")

        # ==================== refine + final ====================
        epool_cm.__exit__(None, None, None)
        tfs_cm.__exit__(None, None, None)
        tfp_cm.__exit__(None, None, None)
        nc.vector.tensor_copy(
            interior(pad_rf_a, 0), x[:, 0, :].rearrange("p (h w) -> p h w", w=HW)
        )
        nc.scalar.copy(
            interior(pad_rf_a, 1), x[:, 1, :].rearrange("p (h w) -> p h w", w=HW)
        )
        src, dst = pad_rf_a, pad_rf_b
        for li in range(3):
            wt = load(f"rf{li}_w", tag="convw")
            bt = load(f"rf{li}_b", tag="convb")
            conv3x3(
                src,
                lambda kb, ky, kx, ob, wt=wt: wt[:, kb, ky, kx, ob * 128 : (ob + 1) * 128],
                2,
                relu_to_pad(dst, bt),
            )
            tap_pad(f"rf{li}", dst)
            src, dst = dst, src
        wt = load("fin_w1", tag="convw")
        bt = load("fin_b1", tag="finb1")

        def fin_writer(ob, f, ps):
            nc.scalar.activation(
                hid[:, 512 * f : 512 * (f + 1)], ps[:], AF.Relu, bias=bt[:, 0:1]
            )

        conv3x3(src, lambda kb, ky, kx, ob, wt=wt: wt[:, kb, ky, kx, :], 1, fin_writer)
        w2f = load("fin_w2", tag="finw2")
        b2f = load("fin_b2", tag="finb2")
        for f in range(2):
            ps = pp.tile([128, 512], F32, tag="ps")
            nc.tensor.matmul(
                ps[0:1, :], w2f[:], hid[:, 512 * f : 512 * (f + 1)], start=True, stop=True
            )
            nc.scalar.activation(
                osb[0:1, 512 * f : 512 * (f + 1)], ps[0:1, :], AF.Identity, bias=b2f[:]
            )
        nc.sync.dma_start(out_d.ap(), osb[:])


    nc.compile()
    return nc


_CACHE = {}


def _get_graph(shapes, taps=()):
    key = tuple(sorted(taps))
    if key not in _CACHE:
        _CACHE[key] = build_graph(shapes, taps)
    return _CACHE[key]


def run(z_pace, z_adrn, params, taps=(), trace=False):
    consts = prep_consts(params)
    zp = _np(z_pace).reshape(B, C, S)
    za = _np(z_adrn).reshape(B, C, S)
    shapes = dict(consts)
    shapes["z_pace"] = np.zeros((128, 2, S), BF)
    shapes["z_adrn"] = np.zeros((128, 2, S), BF)
    nc = _get_graph(shapes, taps)
    in_maps = []
    for b in range(B):
        m = dict(consts)
        m["z_pace"] = np.ascontiguousarray(
            zp[b].reshape(2, 128, S).transpose(1, 0, 2).astype(BF)
        )
        m["z_adrn"] = np.ascontiguousarray(
            za[b].reshape(2, 128, S).transpose(1, 0, 2).astype(BF)
        )
        in_maps.append(m)
    res = run_bass_kernel_spmd(nc, in_maps, core_ids=list(range(B)), trace=trace)
    out = np.stack(
        [
            np.asarray(res.results[b]["out"], np.float32).reshape(1, HW, HW)
            for b in range(B)
        ]
    )
    return out, res


def kernel(z_pace, z_adrn, params):
    out, _ = run(z_pace, z_adrn, params)
    return out


# revision 22
# speedup vs baseline: 1.2022x; 1.2022x over previous
"""Trainium2 Bass kernel for nn_AdaptiveReconstructionTransformer.

Pure data parallel over batch B=8 across 8 NeuronCores (one sample per core,
no collectives). Per-core the network runs out of SBUF with bf16 matmuls
(fp32 PSUM accumulation):

  - 1x1 convs / MLPs / qkv / proj: matmuls with channels on partitions.
  - 3x3 convs: 18 accumulating shifted matmuls over a zero-padded 34x34 image
    kept in SBUF; eval-mode BN folded into weights/bias host-side.
  - dynamic conv: kernel-bank mixing on the PE with a delta-structured
    (4*32 x 32) mixing matrix so mixed weights come out with the contraction
    dim on partitions, ready to be conv lhsT.
  - attention: q/k computed transposed (channel, token) so scores come out as
    S^T[t, s] with t on partitions; exp on the scalar engine (max-subtraction
    skipped: logits are O(10) in f32); V carries an interleaved ones column so
    each AV matmul also yields the softmax denominator row; per-head recip
    rows are broadcast over partitions with a tiny G matmul.
  - the per-head physics bias (pbias) is constant within each softmax row, so
    it cancels and is dropped.
"""

import sys

if "/opt/trn_rl_repo" not in sys.path:
    sys.path.insert(0, "/opt/trn_rl_repo")

from contextlib import ExitStack

import numpy as np
import ml_dtypes

import concourse.bass as bass
import concourse.mybir as mybir
import concourse.tile as tile
from concourse import bacc
from concourse.bass_utils import run_bass_kernel_spmd

BF16 = mybir.dt.bfloat16
F32 = mybir.dt.float32
AF = mybir.ActivationFunctionType
ALU = mybir.AluOpType
AX = mybir.AxisListType

B = 8
C = 256
S = 1024
HW = 32
PW = 34
PS = PW * PW
NH = 8
HD = 32
NK = 4
BN_SCALE = np.float32(1.0 / np.sqrt(1.0 + 1e-5))
EPS_GN = 1e-5

BF = ml_dtypes.bfloat16


def _np(x):
    return np.asarray(x, dtype=np.float32)


def _bf(x):
    return np.ascontiguousarray(np.asarray(x, dtype=np.float32).astype(BF))


def _f(x):
    return np.ascontiguousarray(np.asarray(x, dtype=np.float32))


def lhsT_1x1(Wm):
    """W (O, I) -> (128, I//128, O) with [p, kb, o] = W[o, kb*128+p]."""
    O, I = Wm.shape
    return np.ascontiguousarray(Wm.T.reshape(I // 128, 128, O).transpose(1, 0, 2))


def lhsT_3x3(Wc):
    """W (O, I, 3, 3) -> (128, I//128, 3, 3, O)."""
    O, I = Wc.shape[:2]
    t = Wc.transpose(1, 2, 3, 0)  # (I, ky, kx, O)
    t = t.reshape(I // 128, 128, 3, 3, O).transpose(1, 0, 2, 3, 4)
    return np.ascontiguousarray(t)


def bias_cols(b):
    """(O,) -> (128, O//128) with [p, ob] = b[ob*128+p]."""
    return np.ascontiguousarray(_np(b).reshape(-1, 128).T)


def prep_consts(params):
    cst = {}
    s = BN_SCALE

    fu = params["fusion"]
    for nm in ("pace", "adrn"):
        g = _np(fu[f"{nm}_bn_g"]) * s
        w = _np(fu[f"{nm}_w"])[:, :, 0, 0] * g[:, None]
        bb = _np(fu[f"{nm}_b"]) * g + _np(fu[f"{nm}_bn_b"])
        cst[f"fu_{nm}_w"] = _bf(lhsT_1x1(w))
        cst[f"fu_{nm}_b"] = _f(bias_cols(bb))
    aw = _np(fu["att_w"])[:, :, 0, 0]  # (2, 512)
    ab = _np(fu["att_b"])
    wd = aw[0] - aw[1]
    cst["fu_att_wp"] = _bf(lhsT_1x1(wd[:C][None, :]))  # (128, 2, 1)
    cst["fu_att_wa"] = _bf(lhsT_1x1(wd[C:][None, :]))
    cst["fu_att_bd"] = _f((ab[0] - ab[1]).reshape(1, 1))
    g = _np(fu["ref_bn_g"]) * s
    rw = _np(fu["ref_w"]) * g[:, None, None, None]
    rb = _np(fu["ref_b"]) * g + _np(fu["ref_bn_b"])
    cst["fu_ref_w"] = _bf(lhsT_3x3(rw))
    cst["fu_ref_b"] = _f(bias_cols(rb))

    for li, p in enumerate(params["dyn"]):
        g = _np(p["bn_g"]) * s
        bank = _np(p["bank"]) * g[None, :, None, None, None]  # (NK, O, I, 3, 3)
        t = bank.transpose(0, 2, 3, 4, 1)  # (n, I, ky, kx, O)
        t = t.reshape(NK, 2, 4, 32, 3, 3, C)  # (n, kb, j, ic, ky, kx, o)
        t = t.transpose(1, 2, 0, 3, 4, 5, 6)  # (kb, j, n, ic, ky, kx, o)
        cst[f"dy{li}_bank"] = _bf(t.reshape(2, 4, 128, 9 * C))
        cst[f"dy{li}_cb"] = _f(bias_cols(_np(p["bias"]) * g + _np(p["bn_b"])))
        cst[f"dy{li}_g1w"] = _bf(lhsT_1x1(_np(p["g1_w"])))  # (128, 2, 64)
        cst[f"dy{li}_g1b"] = _f(_np(p["g1_b"]).reshape(64, 1))
        cst[f"dy{li}_g2w"] = _bf(_np(p["g2_w"]).T)  # (64, 4)
        cst[f"dy{li}_g2b"] = _f(_np(p["g2_b"]).reshape(1, 4))

    for li, p in enumerate(params["tf"]):
        qkv = _np(p["qkv_w"])  # (768, 256)
        cst[f"tf{li}_qkw"] = _bf(lhsT_1x1(qkv[: 2 * C]))  # (128, 2, 512)
        cst[f"tf{li}_vw"] = _bf(lhsT_1x1(qkv[2 * C :]))  # (128, 2, 256)
        cst[f"tf{li}_pw"] = _bf(lhsT_1x1(_np(p["proj_w"])))
        cst[f"tf{li}_pb"] = _f(bias_cols(p["proj_b"]))
        cst[f"tf{li}_n1g"] = _f(bias_cols(p["n1_g"]))
        cst[f"tf{li}_n1b"] = _f(bias_cols(p["n1_b"]))
        cst[f"tf{li}_n2g"] = _f(bias_cols(p["n2_g"]))
        cst[f"tf{li}_n2b"] = _f(bias_cols(p["n2_b"]))
        cst[f"tf{li}_w1"] = _bf(lhsT_1x1(_np(p["mlp1_w"])))  # (128, 2, 1024)
        cst[f"tf{li}_b1"] = _f(bias_cols(p["mlp1_b"]))  # (128, 8)
        cst[f"tf{li}_w2"] = _bf(lhsT_1x1(_np(p["mlp2_w"])))  # (128, 8, 256)
        cst[f"tf{li}_b2"] = _f(bias_cols(p["mlp2_b"]))

    for li, p in enumerate(params["refine"]):
        g = _np(p["bn_g"]) * s
        cst[f"rf{li}_w"] = _bf(lhsT_3x3(_np(p["w"]) * g[:, None, None, None]))
        cst[f"rf{li}_b"] = _f(bias_cols(_np(p["b"]) * g + _np(p["bn_b"])))

    fi = params["final"]
    g = _np(fi["bn_g"]) * s
    cst["fin_w1"] = _bf(lhsT_3x3(_np(fi["w1"]) * g[:, None, None, None]))
    cst["fin_b1"] = _f(bias_cols(_np(fi["b1"]) * g + _np(fi["bn_b"])))
    cst["fin_w2"] = _bf(_np(fi["w2"])[:, :, 0, 0].T)  # (128, 1)
    cst["fin_b2"] = _f(_np(fi["b2"]).reshape(1, 1))

    cst["eye32x4"] = _bf(np.tile(np.eye(32, dtype=np.float32), (4, 1)))  # (128, 32)
    cst["ones11"] = _bf(np.ones((1, 1), np.float32))
    gt4 = np.zeros((4, 128), np.float32)
    for n in range(4):
        gt4[n, 32 * n : 32 * (n + 1)] = 1.0
    cst["gt4"] = _bf(gt4)
    ggrp = np.zeros((128, 16), np.float32)
    for p_ in range(128):
        ggrp[p_, p_ // 8] = 1.0 / 8.0
    cst["ggrp"] = _f(ggrp)
    gt16 = np.zeros((16, 128), np.float32)
    for p_ in range(128):
        gt16[p_ // 8, p_] = 1.0
    cst["gt16"] = _f(gt16)
    cst["ones1x128"] = _bf(np.ones((1, 128), np.float32))
    return cst


# ----------------------------------------------------------------------------


def build_graph(shapes, taps=()):
    taps = set(taps)
    nc = bacc.Bacc("TRN2", target_bir_lowering=False, debug=False, num_devices=B)

    def dt_of(a):
        return BF16 if a.dtype == BF else F32

    dram = {}
    for name, arr in shapes.items():
        dram[name] = nc.dram_tensor(name, arr.shape, dt_of(arr), kind="ExternalInput")
    out_d = nc.dram_tensor("out", (1, S), F32, kind="ExternalOutput")
    tap_d = {}

    def add_tap(name, shape, dtype=F32):
        if name in taps:
            tap_d[name] = nc.dram_tensor(
                f"tap_{name}", shape, dtype, kind="ExternalOutput"
            )
            return tap_d[name]
        return None

    with tile.TileContext(nc) as tc, ExitStack() as ctx:
        acts = ctx.enter_context(tc.tile_pool(name="acts", bufs=1))
        wp = ctx.enter_context(tc.tile_pool(name="wp", bufs=2))
        tp = ctx.enter_context(tc.tile_pool(name="tp", bufs=3))
        pp = ctx.enter_context(tc.tile_pool(name="pp", bufs=2, space="PSUM"))
        pps = ctx.enter_context(tc.tile_pool(name="pps", bufs=3, space="PSUM"))
        convA_cm = tc.tile_pool(name="convA", bufs=1)
        convA = convA_cm.__enter__()
        wmixp_cm = tc.tile_pool(name="wmixp", bufs=1)
        wmixp = wmixp_cm.__enter__()
        wbank_cm = tc.tile_pool(name="wbank", bufs=8)
        wbank = wbank_cm.__enter__()

        def load(name, pool=wp, tag=None):
            t = pool.tile(list(shapes[name].shape), dt_of(shapes[name]), tag=tag or name)
            nc.sync.dma_start(t[:], dram[name].ap())
            return t

        # persistent activations
        x = acts.tile([128, 2, S], F32, tag="x")
        pad_a = convA.tile([128, 2, PS], BF16, tag="pad_a")
        pad_b = convA.tile([128, 2, PS], BF16, tag="pad_b")

        nc.vector.memset(pad_a[:], 0.0)
        nc.vector.memset(pad_b[:], 0.0)

        eye32x4 = load("eye32x4", pool=acts)
        ones11 = load("ones11", pool=acts)
        gt4 = load("gt4", pool=acts)
        ggrp = load("ggrp", pool=acts)
        gt16 = load("gt16", pool=acts)
        ones1x128 = load("ones1x128", pool=acts)
        epsgn = acts.tile([16, 1], F32, tag="epsgn")
        nc.vector.memset(epsgn[:], EPS_GN)

        def interior(padt, kb):
            v = padt[:, kb, :].rearrange("p (h w) -> p h w", w=PW)
            return v[:, 1 : 1 + HW, 1 : 1 + HW]

        def pad_img(padt, kb):
            return padt[:, kb, :].rearrange("p (h w) -> p h w", w=PW)

        def conv3x3(src_pad, lhsT_fn, n_ob, writer):
            for ob in range(n_ob):
                for f in range(2):
                    ps = pp.tile([128, 512], F32, tag="ps")
                    first = True
                    for kb in range(2):
                        xv = pad_img(src_pad, kb)
                        for ky in range(3):
                            for kx in range(3):
                                nc.tensor.matmul(
                                    ps[:],
                                    lhsT_fn(kb, ky, kx, ob),
                                    xv[:, ky + 16 * f : ky + 16 * f + 16, kx : kx + 32],
                                    start=first,
                                    stop=(kb == 1 and ky == 2 and kx == 2),
                                )
                                first = False
                    writer(ob, f, ps)

        def relu_to_pad(dst_pad, bias_t):
            def w(ob, f, ps):
                dv = pad_img(dst_pad, ob)[:, 1 + 16 * f : 1 + 16 * f + 16, 1 : 1 + HW]
                nc.vector.tensor_scalar(
                    out=dv,
                    in0=ps[:].rearrange("p (a b) -> p a b", b=32),
                    scalar1=bias_t[:, ob : ob + 1],
                    scalar2=0.0,
                    op0=ALU.add,
                    op1=ALU.max,
                )
            return w

        def tap_pad(name, padt, nb=2):
            t = add_tap(name, (128, nb, HW * HW), BF16)
            if t is not None:
                for kb in range(nb):
                    nc.sync.dma_start(
                        t.ap().rearrange("p k (h w) -> p k h w", w=HW)[:, kb],
                        interior(padt, kb),
                    )

        def tap_x(name):
            t = add_tap(name, (128, 2, S))
            if t is not None:
                nc.sync.dma_start(t.ap(), x[:])

        # ==================== fusion ====================
        fus_cm = tc.tile_pool(name="fus", bufs=1)
        fus = fus_cm.__enter__()
        zin_p = load("z_pace", pool=fus)
        zin_a = load("z_adrn", pool=fus)
        zp_sb = fus.tile([128, 2, S], BF16, tag="zp")
        za_sb = fus.tile([128, 2, S], BF16, tag="za")
        w0sb = fus.tile([1, S], BF16, tag="w0sb")

        for nm, zin, zout in (("pace", zin_p, zp_sb), ("adrn", zin_a, za_sb)):
            wt = load(f"fu_{nm}_w")
            bt = load(f"fu_{nm}_b")
            for ob in range(2):
                for f in range(2):
                    ps = pp.tile([128, 512], F32, tag="ps")
                    for kb in range(2):
                        nc.tensor.matmul(
                            ps[:],
                            wt[:, kb, ob * 128 : (ob + 1) * 128],
                            zin[:, kb, 512 * f : 512 * (f + 1)],
                            start=(kb == 0),
                            stop=(kb == 1),
                        )
                    nc.scalar.activation(
                        zout[:, ob, 512 * f : 512 * (f + 1)],
                        ps[:],
                        AF.Relu,
                        bias=bt[:, ob : ob + 1],
                    )

        watp = load("fu_att_wp")
        wata = load("fu_att_wa")
        batd = load("fu_att_bd")
        for f in range(2):
            psg = pp.tile([128, 512], F32, tag="ps")
            k = 0
            for wt_, zt_ in ((watp, zp_sb), (wata, za_sb)):
                for kb in range(2):
                    nc.tensor.matmul(
                        psg[0:1, :],
                        wt_[:, kb, :],
                        zt_[:, kb, 512 * f : 512 * (f + 1)],
                        start=(k == 0),
                        stop=(k == 3),
                    )
                    k += 1
            nc.scalar.activation(
                w0sb[0:1, 512 * f : 512 * (f + 1)],
                psg[0:1, :],
                AF.Sigmoid,
                bias=batd[0:1, :],
            )
        for f in range(2):
            psw = pp.tile([128, 512], F32, tag="ps")
            nc.tensor.matmul(
                psw[:], ones1x128[:], w0sb[0:1, 512 * f : 512 * (f + 1)],
                start=True, stop=True,
            )
            for kb in range(2):
                t1 = tp.tile([128, 512], BF16, tag="gate_t1")
                nc.vector.tensor_sub(
                    t1[:],
                    zp_sb[:, kb, 512 * f : 512 * (f + 1)],
                    za_sb[:, kb, 512 * f : 512 * (f + 1)],
                )
                nc.vector.tensor_mul(t1[:], t1[:], psw[:])
                nc.vector.tensor_add(
                    pad_img(pad_a, kb)[:, 1 + 16 * f : 1 + 16 * f + 16, 1 : 1 + HW],
                    t1[:].rearrange("p (a b) -> p a b", b=32),
                    za_sb[:, kb, 512 * f : 512 * (f + 1)].rearrange(
                        "p (a b) -> p a b", b=32
                    ),
                )
        tap_pad("zf", pad_a)

        wt = load("fu_ref_w", tag="convw")
        bt = load("fu_ref_b", tag="convb")
        conv3x3(
            pad_a,
            lambda kb, ky, kx, ob, wt=wt: wt[:, kb, ky, kx, ob * 128 : (ob + 1) * 128],
            2,
            relu_to_pad(pad_b, bt),
        )
        fus_cm.__exit__(None, None, None)
        tap_pad("fus", pad_b)

        # ==================== dynamic convs ====================
        src, dst = pad_b, pad_a
        for li in range(3):
            g1w = load(f"dy{li}_g1w", tag="g1w")
            g1b = load(f"dy{li}_g1b", tag="g1b")
            g2w = load(f"dy{li}_g2w", tag="g2w")
            g2b = load(f"dy{li}_g2b", tag="g2b")
            cb = load(f"dy{li}_cb", tag="cb")

            gap = tp.tile([128, 2], F32, tag="gap")
            for kb in range(2):
                nc.vector.reduce_sum(gap[:, kb : kb + 1], interior(src, kb), axis=AX.XY)
            gapb = tp.tile([128, 2], BF16, tag="gapb")
            nc.vector.tensor_scalar_mul(gapb[:], gap[:], 1.0 / float(S))

            ps1 = pp.tile([128, 512], F32, tag="ps")
            for kb in range(2):
                nc.tensor.matmul(
                    ps1[0:64, 0:1], g1w[:, kb, :], gapb[:, kb : kb + 1],
                    start=(kb == 0), stop=(kb == 1),
                )
            h1 = tp.tile([64, 1], BF16, tag="h1")
            nc.scalar.activation(h1[:], ps1[0:64, 0:1], AF.Relu, bias=g1b[:])

            ps2 = pp.tile([128, 512], F32, tag="ps")
            nc.tensor.matmul(ps2[0:1, 0:4], h1[:], g2w[:], start=True, stop=True)
            lg = tp.tile([1, 4], F32, tag="lg")
            nc.vector.tensor_add(lg[:], ps2[0:1, 0:4], g2b[:])
            nc.scalar.activation(lg[:], lg[:], AF.Exp)
            lsum = tp.tile([1, 1], F32, tag="lsum")
            nc.vector.reduce_sum(lsum[:], lg[:], axis=AX.X)
            nc.vector.reciprocal(lsum[:], lsum[:])
            mixn = tp.tile([1, 4], BF16, tag="mixn")
            nc.vector.tensor_scalar_mul(mixn[:], lg[:], lsum[:])
            if f"mix{li}" in taps:
                t = add_tap(f"mix{li}", (1, 4))
                mixf = tp.tile([1, 4], F32, tag="mixf")
                nc.vector.tensor_copy(mixf[:], mixn[:])
                nc.sync.dma_start(t.ap(), mixf[:])

            psT = pp.tile([128, 512], F32, tag="ps")
            nc.tensor.matmul(psT[0:4, 0:1], mixn[:], ones11[:], start=True, stop=True)
            mixT = tp.tile([4, 1], BF16, tag="mixT")
            nc.vector.tensor_copy(mixT[:], psT[0:4, 0:1])
            psbc = pp.tile([128, 512], F32, tag="ps")
            nc.tensor.matmul(psbc[:, 0:1], gt4[:], mixT[:], start=True, stop=True)
            delta = tp.tile([128, 32], BF16, tag="delta")
            nc.vector.tensor_scalar_mul(delta[:], eye32x4[:], psbc[:, 0:1])

            wmix = wmixp.tile([128, 2, 9 * C], BF16, tag="wmix")
            for kb in range(2):
                bcs = []
                for j in range(4):
                    bc = wbank.tile([128, 9 * C], BF16, tag="bank")
                    nc.sync.dma_start(bc[:], dram[f"dy{li}_bank"].ap()[kb, j])
                    bcs.append(bc)
                off = 0
                while off < 9 * C:
                    cw = min(512, 9 * C - off)
                    psW = pp.tile([128, 512], F32, tag="ps")
                    for j in range(4):
                        nc.tensor.matmul(
                            psW[32 * j : 32 * (j + 1), :cw],
                            delta[:],
                            bcs[j][:, off : off + cw],
                            start=True,
                            stop=True,
                            tile_position=(0, 32 * j),
                        )
                    nc.vector.tensor_copy(wmix[:, kb, off : off + cw], psW[:, :cw])
                    off += cw

            conv3x3(
                src,
                lambda kb, ky, kx, ob, wmix=wmix: wmix[
                    :, kb, (ky * 3 + kx) * C + ob * 128 : (ky * 3 + kx) * C + (ob + 1) * 128
                ],
                2,
                relu_to_pad(dst, cb),
            )
            tap_pad(f"dyn{li}", dst)
            src, dst = dst, src

        # dyn0: b->a, dyn1: a->b, dyn2: b->a  =>  result in pad_a (== src)
        for kb in range(2):
            nc.vector.tensor_copy(
                x[:, kb, :].rearrange("p (h w) -> p h w", w=HW), interior(src, kb)
            )
        tap_x("x0")
        wbank_cm.__exit__(None, None, None)
        wmixp_cm.__exit__(None, None, None)
        convA_cm.__exit__(None, None, None)

        # ==================== transformer ====================
        convB = ctx.enter_context(tc.tile_pool(name="convB", bufs=1))
        pad_rf_a = convB.tile([128, 2, PS], BF16, tag="pad_a2")
        pad_rf_b = convB.tile([128, 2, PS], BF16, tag="pad_b2")
        hid = convB.tile([128, S], BF16, tag="hid")
        osb = convB.tile([1, S], F32, tag="osb")
        nc.vector.memset(pad_rf_a[:], 0.0)
        nc.vector.memset(pad_rf_b[:], 0.0)
        tfp_cm = tc.tile_pool(name="tfp", bufs=1)
        tfp = tfp_cm.__enter__()
        tfs_cm = tc.tile_pool(name="tfs", bufs=3)
        tfs = tfs_cm.__enter__()
        epool_cm = tc.tile_pool(name="epool", bufs=4)
        epool = epool_cm.__enter__()
        gnx = tfp.tile([128, 2, S], BF16, tag="gnx")
        qkT = tfp.tile([128, 4, S], BF16, tag="qkT")
        vaug = tfp.tile([128, 8, 8 * 64], BF16, tag="vaug")
        attnT = tfp.tile([128, 2, S], BF16, tag="attnT")
        hsb = tfp.tile([128, 8, S], BF16, tag="hsb")
        nc.vector.memset(
            vaug[:].rearrange("p tb (h e) -> p tb h e", e=64)[:, :, :, 32:64], 1.0
        )

        def groupnorm(gamma_t, beta_t, out_t):
            st4 = tp.tile([128, 4], F32, tag="st4")
            for kb in range(2):
                stats = tp.tile([128, 2, 6], F32, tag="gnstats")
                for i in range(2):
                    nc.vector.bn_stats(stats[:, i, :], x[:, kb, 512 * i : 512 * (i + 1)])
                nc.vector.bn_aggr(st4[:, 2 * kb : 2 * kb + 2], stats[:])
            tmp1 = tp.tile([128, 2], F32, tag="gn_t1")
            m_cols = st4[:].rearrange("p (k two) -> p k two", two=2)[:, :, 0]
            v_cols = st4[:].rearrange("p (k two) -> p k two", two=2)[:, :, 1]
            nc.vector.tensor_mul(tmp1[:], m_cols, m_cols)
            nc.vector.tensor_add(v_cols, v_cols, tmp1[:])
            psG = pp.tile([128, 512], F32, tag="ps")
            nc.tensor.matmul(psG[0:16, 0:4], ggrp[:], st4[:], start=True, stop=True)
            gst = tp.tile([16, 4], F32, tag="gst")
            nc.scalar.copy(gst[:], psG[0:16, 0:4])
            rs4 = tp.tile([16, 4], F32, tag="rs4")  # [var0, var1, m0, m1] -> rstd
            gm = gst[:].rearrange("p (k two) -> p k two", two=2)[:, :, 0]
            gv = gst[:].rearrange("p (k two) -> p k two", two=2)[:, :, 1]
            nc.vector.tensor_mul(rs4[:, 0:2], gm, gm)
            nc.vector.tensor_sub(rs4[:, 0:2], gv, rs4[:, 0:2])
            nc.scalar.copy(rs4[:, 2:4], gm)
            nc.scalar.activation(rs4[:, 0:2], rs4[:, 0:2], AF.Sqrt, bias=epsgn[:])
            nc.vector.reciprocal(rs4[:, 0:2], rs4[:, 0:2])
            psB = pp.tile([128, 512], F32, tag="ps")
            nc.tensor.matmul(psB[:, 0:4], gt16[:], rs4[:], start=True, stop=True)
            scl = tp.tile([128, 2], F32, tag="gn_scl")
            bsh = tp.tile([128, 2], F32, tag="gn_bsh")
            nc.vector.tensor_mul(scl[:], psB[:, 0:2], gamma_t[:])
            nc.vector.tensor_mul(bsh[:], psB[:, 2:4], scl[:])
            nc.vector.tensor_sub(bsh[:], beta_t[:], bsh[:])
            for kb in range(2):
                nc.scalar.activation(
                    out_t[:, kb, :], x[:, kb, :], AF.Identity,
                    bias=bsh[:, kb : kb + 1], scale=scl[:, kb : kb + 1],
                )

        for li in range(4):
            n1g = load(f"tf{li}_n1g", tag="n1g")
            n1b = load(f"tf{li}_n1b", tag="n1b")
            groupnorm(n1g, n1b, gnx)
            if li == 0 and "gnx0" in taps:
                t = add_tap("gnx0", (128, 2, S), BF16)
                nc.sync.dma_start(t.ap(), gnx[:])

            qkw = load(f"tf{li}_qkw", tag="qkw")
            for mb in range(4):
                for f in range(2):
                    ps = pp.tile([128, 512], F32, tag="ps")
                    for kb in range(2):
                        nc.tensor.matmul(
                            ps[:],
                            qkw[:, kb, mb * 128 : (mb + 1) * 128],
                            gnx[:, kb, 512 * f : 512 * (f + 1)],
                            start=(kb == 0),
                            stop=(kb == 1),
                        )
                    nc.vector.tensor_copy(qkT[:, mb, 512 * f : 512 * (f + 1)], ps[:])

            vw = load(f"tf{li}_vw", tag="vw")
            ver = vaug[:].rearrange("p tb (h e) -> p tb h e", e=64)
            for sb in range(8):
                ps = pp.tile([128, 512], F32, tag="ps")
                for kb in range(2):
                    nc.tensor.matmul(
                        ps[:, 0:256],
                        gnx[:, kb, sb * 128 : (sb + 1) * 128],
                        vw[:, kb, :],
                        start=(kb == 0),
                        stop=(kb == 1),
                    )
                nc.vector.tensor_copy(
                    ver[:, sb, :, 0:32],
                    ps[:, 0:256].rearrange("p (h e) -> p h e", e=32),
                )

            # attention, grouped by (q/k block, s-chunk): the 4 heads of a
            # block run their score matmuls interleaved across the 4 PE row
            # strips (concurrent in hardware); AV matmuls run col-packed two
            # heads at a time at array columns 0 and 64. Row 32/96 of each AV
            # psum is the softmax denominator (ones column in V); denominators
            # are broadcast over partitions with K=1 mask matmuls and inverted
            # with the fast approximate reciprocal.
            scale = float(HD) ** -0.5
            E_tiles = {}

            def emit_scores(kbh, sc):
                # two double-width E tiles: Epair[jp] holds heads 2*jp and 2*jp+1
                Es = [epool.tile([128, 8, 2, 512], BF16, tag="E", name=f"E_{kbh}_{sc}_{jj}") for jj in range(2)]
                E_tiles[(kbh, sc)] = Es
                for tb in range(8):
                    for jp in range(2):
                        psS = pps.tile([128, 1024], F32, tag="psS")
                        for idx in range(2):
                            j = 2 * jp + idx
                            nc.tensor.matmul(
                                psS[:, 512 * idx : 512 * (idx + 1)],
                                qkT[32 * j : 32 * (j + 1), 2 + kbh, tb * 128 : (tb + 1) * 128],
                                qkT[32 * j : 32 * (j + 1), kbh, 512 * sc : 512 * (sc + 1)],
                                start=True,
                                stop=True,
                                tile_position=(32 * j, 0),
                            )
                        if jp == 1 and tb % 2 == 0:
                            # Schraudolph-style exp on DVE+GpSimd to offload
                            # the scalar engine: bitcast(int32(A*scale*x + B))
                            # ~= exp(scale*x) to within +-3%; softmax
                            # normalization cancels the common-mode part.
                            yi = tfs.tile([128, 1024], mybir.dt.int32, tag="yi")
                            nc.vector.tensor_scalar(
                                out=yi[:],
                                in0=psS[:],
                                scalar1=12102203.1616 * scale,
                                scalar2=1064866805.0,
                                op0=ALU.mult,
                                op1=ALU.add,
                            )
                            nc.vector.tensor_copy(
                                Es[jp][:, tb],
                                yi[:].bitcast(F32).rearrange("p (a b) -> p a b", a=2),
                            )
                        else:
                            nc.scalar.activation(
                                Es[jp][:, tb], psS[:].rearrange("p (a b) -> p a b", a=2),
                                AF.Exp, scale=scale,
                            )

            def emit_av(kbh, sc):
                Es = E_tiles.pop((kbh, sc))
                for pair in ((0, 1), (2, 3)):
                    psA = pp.tile([128, 512], F32, tag="ps")
                    for tb in range(8):
                        for idx, j in enumerate(pair):
                            h = 4 * kbh + j
                            nc.tensor.matmul(
                                psA[64 * idx : 64 * (idx + 1), :],
                                vaug[:, tb, 64 * h : 64 * (h + 1)],
                                Es[j // 2][:, tb, j % 2],
                                start=(tb == 0),
                                stop=(tb == 7),
                                tile_position=(0, 64 * idx),
                            )
                    # rows [32:64) and [96:128) hold 32 copies of each head's
                    # softmax denominator (ones block in V); invert the whole
                    # tile in one pass (numerator lanes unused) and normalize.
                    rp = tfs.tile([128, 512], F32, tag="rp")
                    nc.vector.reciprocal_approx_fast(rp[:], psA[:])
                    for idx, j in enumerate(pair):
                        nc.vector.tensor_mul(
                            attnT[32 * j : 32 * (j + 1), kbh, 512 * sc : 512 * (sc + 1)],
                            psA[64 * idx : 64 * idx + 32, :],
                            rp[64 * idx + 32 : 64 * (idx + 1), :],
                        )

            groups = [(kbh, sc) for kbh in range(2) for sc in range(2)]
            emit_scores(*groups[0])
            for i, g in enumerate(groups):
                if i + 1 < len(groups):
                    emit_scores(*groups[i + 1])
                emit_av(*g)

            pw = load(f"tf{li}_pw", tag="pw")
            pb = load(f"tf{li}_pb", tag="pb")
            for ob in range(2):
                for f in range(2):
                    ps = pp.tile([128, 512], F32, tag="ps")
                    for kb in range(2):
                        nc.tensor.matmul(
                            ps[:],
                            pw[:, kb, ob * 128 : (ob + 1) * 128],
                            attnT[:, kb, 512 * f : 512 * (f + 1)],
                            start=(kb == 0),
                            stop=(kb == 1),
                        )
                    tb_ = tp.tile([128, 512], F32, tag="resid_t")
                    nc.scalar.activation(
                        tb_[:], ps[:], AF.Identity, bias=pb[:, ob : ob + 1]
                    )
                    nc.vector.tensor_add(
                        x[:, ob, 512 * f : 512 * (f + 1)],
                        x[:, ob, 512 * f : 512 * (f + 1)],
                        tb_[:],
                    )
            tap_x(f"attn{li}")

            n2g = load(f"tf{li}_n2g", tag="n2g")
            n2b = load(f"tf{li}_n2b", tag="n2b")
            groupnorm(n2g, n2b, gnx)
            w1 = load(f"tf{li}_w1", tag="w1")
            b1 = load(f"tf{li}_b1", tag="b1")
            for mb in range(8):
                for f in range(2):
                    ps = pp.tile([128, 512], F32, tag="ps")
                    for kb in range(2):
                        nc.tensor.matmul(
                            ps[:],
                            w1[:, kb, mb * 128 : (mb + 1) * 128],
                            gnx[:, kb, 512 * f : 512 * (f + 1)],
                            start=(kb == 0),
                            stop=(kb == 1),
                        )
                    nc.scalar.activation(
                        hsb[:, mb, 512 * f : 512 * (f + 1)], ps[:], AF.Gelu,
                        bias=b1[:, mb : mb + 1],
                    )
            w2 = load(f"tf{li}_w2", tag="w2")
            b2 = load(f"tf{li}_b2", tag="b2")
            for ob in range(2):
                for f in range(2):
                    ps = pp.tile([128, 512], F32, tag="ps")
                    for kb in range(8):
                        nc.tensor.matmul(
                            ps[:],
                            w2[:, kb, ob * 128 : (ob + 1) * 128],
                            hsb[:, kb, 512 * f : 512 * (f + 1)],
                            start=(kb == 0),
                            stop=(kb == 7),
                        )
                    tb_ = tp.tile([128, 512], F32, tag="resid_t")
                    nc.scalar.activation(
                        tb_[:], ps[:], AF.Identity, bias=b2[:, ob : ob + 1]
                    )
                    nc.vector.tensor_add(
                        x[:, ob, 512 * f : 512 * (f + 1)],
                        x[:, ob, 512 * f : 512 * (f + 1)],
                        tb_[:],
                    )
            tap_x(f"tf{li}")

        # ==================== refine + final ====================
        epool_cm.__exit__(None, None, None)
        tfs_cm.__exit__(None, None, None)
        tfp_cm.__exit__(None, None, None)
        nc.vector.tensor_copy(
            interior(pad_rf_a, 0), x[:, 0, :].rearrange("p (h w) -> p h w", w=HW)
        )
        nc.scalar.copy(
            interior(pad_rf_a, 1), x[:, 1, :].rearrange("p (h w) -> p h w", w=HW)
        )
        src, dst = pad_rf_a, pad_rf_b
        for li in range(3):
            wt = load(f"rf{li}_w", tag="convw")
            bt = load(f"rf{li}_b", tag="convb")
            conv3x3(
                src,
                lambda kb, ky, kx, ob, wt=wt: wt[:, kb, ky, kx, ob * 128 : (ob + 1) * 128],
                2,
                relu_to_pad(dst, bt),
            )
            tap_pad(f"rf{li}", dst)
            src, dst = dst, src
        wt = load("fin_w1", tag="convw")
        bt = load("fin_b1", tag="finb1")

        def fin_writer(ob, f, ps):
            nc.scalar.activation(
                hid[:, 512 * f : 512 * (f + 1)], ps[:], AF.Relu, bias=bt[:, 0:1]
            )

        conv3x3(src, lambda kb, ky, kx, ob, wt=wt: wt[:, kb, ky, kx, :], 1, fin_writer)
        w2f = load("fin_w2", tag="finw2")
        b2f = load("fin_b2", tag="finb2")
        for f in range(2):
            ps = pp.tile([128, 512], F32, tag="ps")
            nc.tensor.matmul(
                ps[0:1, :], w2f[:], hid[:, 512 * f : 512 * (f + 1)], start=True, stop=True
            )
            nc.scalar.activation(
                osb[0:1, 512 * f : 512 * (f + 1)], ps[0:1, :], AF.Identity, bias=b2f[:]
            )
        nc.sync.dma_start(out_d.ap(), osb[:])


    nc.compile()
    return nc


_CACHE = {}


def _get_graph(shapes, taps=()):
    key = tuple(sorted(taps))
    if key not in _CACHE:
        _CACHE[key] = build_graph(shapes, taps)
    return _CACHE[key]


def run(z_pace, z_adrn, params, taps=(), trace=False):
    consts = prep_consts(params)
    zp = _np(z_pace).reshape(B, C, S)
    za = _np(z_adrn).reshape(B, C, S)
    shapes = dict(consts)
    shapes["z_pace"] = np.zeros((128, 2, S), BF)
    shapes["z_adrn"] = np.zeros((128, 2, S), BF)
    nc = _get_graph(shapes, taps)
    in_maps = []
    for b in range(B):
        m = dict(consts)
        m["z_pace"] = np.ascontiguousarray(
            zp[b].reshape(2, 128, S).transpose(1, 0, 2).astype(BF)
        )
        m["z_adrn"] = np.ascontiguousarray(
            za[b].reshape(2, 128, S).transpose(1, 0, 2).astype(BF)
        )
        in_maps.append(m)
    res = run_bass_kernel_spmd(nc, in_maps, core_ids=list(range(B)), trace=trace)
    out = np.stack(
        [
            np.asarray(res.results[b]["out"], np.float32).reshape(1, HW, HW)
            for b in range(B)
        ]
    )
    return out, res


def kernel(z_pace, z_adrn, params):
    out, _ = run(z_pace, z_adrn, params)
    return out
